# revision 27
# baseline (speedup 1.0000x reference)
"""Grouped matmul (MoE routing) kernel for Trainium2, 8 NeuronCores.

Problem: y[t] = x[t] @ weight[e].T for tokens t in [starts[e], offs[e]),
with x [4096, 2048] f32, weight [8, 1024, 2048] f32, offs [8] int32
(cumulative group ends). Output [4096, 1024] f32; tokens >= offs[-1] -> 0.

Strategy: expert-parallel. Routing is done host-side (offs is a host
numpy array): core e receives its expert's token slice, transposed and
zero-padded to P rows (x_e^T [K, P]), plus its expert's transposed
weight (w_e^T [K, N]). Each core runs a dense [P,K]x[K,N] matmul; the
host scatters per-core outputs back into the full [T, N] result.

Default path (v6, bf16): single fused pre-tiled input tensor
[128, K/128, P+N] per core ([x|w] per k-tile), ramped slab DMAs on both
HWDGE rings, k-outer accumulation over all 8 PSUM banks while the input
streams, then bank-major k-chains with immediate per-bank PSUM->SBUF
bf16 copy + store (host upcasts y to f32). Per-core floor is the PE
matmul stream: 128 MMs x 216ns = 27.7us (bf16 streams 1 col/cycle;
fp8 fails the 2e-2 accuracy gate, measured 3.8% rel err).

Matmul dtype modes (GMM_MODE env): fp32 (exact, 4 cyc/row), fp32r
(1 cyc/row at N=512), bf16 (1 cyc/row, half DMA), bf16x3 (hi/lo split,
near-fp32 accuracy, 3x bf16 compute).
"""

import math
import os
import sys

for _p in ("/opt/pypackages", "/opt/trn_rl_repo"):
    if _p not in sys.path:
        sys.path.insert(0, _p)

import numpy as np

E, K, N, T = 8, 2048, 1024, 4096
NCORES = 8
KT = 128  # contraction tile (PE partition dim)
NT = 512  # psum free-dim chunk (one PSUM bank of f32)
MB = 512  # m-block rows kept resident in SBUF at once

MODE = os.environ.get("GMM_MODE", "bf16")
TRACE = bool(int(os.environ.get("GMM_TRACE", "0")))
CSPL = int(os.environ.get("GMM_CSPL", "1"))  # column-group split in v6
WARM = int(os.environ.get("GMM_WARM", "0"))  # PE warm-up dummy matmuls

_nc_cache = {}
last_result = None  # BassKernelResults of the most recent run (for test.py)


def _dtypes(mode):
    from concourse import mybir

    if mode == "fp32":
        return mybir.dt.float32, np.float32
    if mode == "fp32r":
        return mybir.dt.float32r, np.float32
    import ml_dtypes

    return mybir.dt.bfloat16, np.dtype(ml_dtypes.bfloat16)


def _build_v4(P, mode):
    """v3 + host-pre-tiled inputs ([128, K/128, cols] layout -> 8KB DMA
    runs), KS=4, all stores on SWDGE. Single-tensor modes only."""
    import concourse.tile as tile
    from concourse import bacc, mybir

    f32 = mybir.dt.float32
    mmdt, _ = _dtypes(mode)

    KS = 4
    n_slab = K // (128 * KS)  # 4
    n_k = K // 128
    n_m = P // 128
    assert n_m <= 8
    n_half = N // 2

    nc = bacc.Bacc(
        "TRN2", target_bir_lowering=False, debug=False, num_devices=NCORES
    )

    w0 = nc.dram_tensor("wT0", [128, n_k, n_half], mmdt, kind="ExternalInput").ap()
    w1 = nc.dram_tensor("wT1", [128, n_k, n_half], mmdt, kind="ExternalInput").ap()
    xt = nc.dram_tensor("xTt", [128, n_k, P], mmdt, kind="ExternalInput").ap()
    y = nc.dram_tensor("y", [P, N], f32, kind="ExternalOutput").ap()

    WKS = 2  # w slab k-subtiles (finer pacing near stream end)
    n_wslab = K // (128 * WKS)

    with tile.TileContext(nc) as tc:
        with (
            tc.tile_pool(name="w0", bufs=n_wslab) as w0pool,
            tc.tile_pool(name="w1", bufs=n_wslab) as w1pool,
            tc.tile_pool(name="x", bufs=n_slab) as xpool,
            tc.tile_pool(name="ps", bufs=8, space="PSUM") as pspool,
            tc.tile_pool(name="o", bufs=8) as opool,
        ):
            # Balanced rings (~6.3MB each), w slabs arriving in k order:
            # ring A (sync): w0/w1 slabs k 0..11; ring B (scalar): x, then
            # w0/w1 slabs k 12..15 (the final MM chain's inputs).
            w0_slabs = [None] * n_wslab
            w1_slabs = [None] * n_wslab
            x_slabs = []
            for s in range(n_wslab - 2):
                ks = slice(s * WKS, (s + 1) * WKS)
                t = w0pool.tile([128, WKS, n_half], mmdt, tag="w0", name=f"w0s{s}")
                nc.sync.dma_start(t[:], w0[:, ks, :])
                w0_slabs[s] = t
                t = w1pool.tile([128, WKS, n_half], mmdt, tag="w1", name=f"w1s{s}")
                nc.sync.dma_start(t[:], w1[:, ks, :])
                w1_slabs[s] = t
            for s in range(n_slab):
                ks = slice(s * KS, (s + 1) * KS)
                t = xpool.tile([128, KS, P], mmdt, tag="x", name=f"xs{s}")
                nc.scalar.dma_start(t[:], xt[:, ks, :])
                x_slabs.append(t)
            for s in range(n_wslab - 2, n_wslab):
                ks = slice(s * WKS, (s + 1) * WKS)
                t = w0pool.tile([128, WKS, n_half], mmdt, tag="w0", name=f"w0s{s}")
                nc.scalar.dma_start(t[:], w0[:, ks, :])
                w0_slabs[s] = t
                t = w1pool.tile([128, WKS, n_half], mmdt, tag="w1", name=f"w1s{s}")
                nc.scalar.dma_start(t[:], w1[:, ks, :])
                w1_slabs[s] = t

            ps_tiles = [
                pspool.tile([128, n_half], f32, tag="ps", name=f"ps{h}_{i}")
                for h in range(2)
                for i in range(n_m)
            ]
            w_halves = [w0_slabs, w1_slabs]
            for k in range(n_k):
                ws, wj = divmod(k, WKS)
                xs, xj = divmod(k, KS)
                for h in range(2):
                    for mi in range(n_m):
                        nc.tensor.matmul(
                            ps_tiles[h * n_m + mi][:, :],
                            x_slabs[xs][:, xj, mi * 128 : (mi + 1) * 128],
                            w_halves[h][ws][:, wj, :],
                            start=(k == 0),
                            stop=(k == n_k - 1),
                        )
            for h in range(2):
                for mi in range(n_m):
                    ot = opool.tile(
                        [128, n_half], f32, tag="o", name=f"o{h}_{mi}"
                    )
                    nc.vector.tensor_copy(ot[:], ps_tiles[h * n_m + mi][:])
                    eng = nc.sync if (h * n_m + mi) % 2 == 0 else nc.scalar
                    eng.dma_start(
                        y[
                            mi * 128 : (mi + 1) * 128,
                            h * n_half : (h + 1) * n_half,
                        ],
                        ot[:],
                    )

    nc.compile()
    return nc


def _build_v5(P, mode):
    """bf16-first redesign from trace analysis of v4:
    - ramped slab sizes (tiny k0/k1 slabs -> early first MM, big tail slabs
      -> few DIRECT2D descriptor-gen ops, ~0.7us each on the sequencer)
    - k-outer / m-mid / h-inner MM order: consecutive MM pairs share the
      same stationary lhsT tile (chance for LDWEIGHTS elision) and slab
      consumption tracks DMA arrival order
    - output stored as bf16 (host upcasts): halves store bytes, DVE 2x
    - copies split DVE/ACT, stores split sync/scalar, emitted per-bank so
      stores overlap the final MMs of later banks."""
    import concourse.tile as tile
    from concourse import bacc, mybir

    f32 = mybir.dt.float32
    mmdt, _ = _dtypes(mode)
    assert mode == "bf16"

    n_k = K // 128  # 16
    n_m = P // 128  # 4
    assert n_m <= 4
    n_half = N // 2  # 512

    # slab ramp: k-tile counts per DMA slab
    RAMP = (1, 1, 2, 2, 4, 3, 3)
    assert sum(RAMP) == n_k
    K_SPLIT = 10  # phase A: k-outer 0..K_SPLIT-1; phase B: bank-major rest

    nc = bacc.Bacc(
        "TRN2", target_bir_lowering=False, debug=False, num_devices=NCORES
    )

    w0 = nc.dram_tensor("wT0", [128, n_k, n_half], mmdt, kind="ExternalInput").ap()
    w1 = nc.dram_tensor("wT1", [128, n_k, n_half], mmdt, kind="ExternalInput").ap()
    xt = nc.dram_tensor("xTt", [128, n_k, P], mmdt, kind="ExternalInput").ap()
    y = nc.dram_tensor("y", [P, N], mmdt, kind="ExternalOutput").ap()

    n_slab = len(RAMP)
    bounds = []
    k0 = 0
    for r in RAMP:
        bounds.append((k0, k0 + r))
        k0 += r

    with tile.TileContext(nc) as tc:
        with (
            tc.tile_pool(name="w0", bufs=n_slab) as w0pool,
            tc.tile_pool(name="w1", bufs=n_slab) as w1pool,
            tc.tile_pool(name="x", bufs=n_slab) as xpool,
            tc.tile_pool(name="ps", bufs=8, space="PSUM") as pspool,
            tc.tile_pool(name="o", bufs=8) as opool,
        ):
            w0_slabs, w1_slabs, x_slabs = [], [], []
            # interleaved issue tracking consumption order: ring A (sync)
            # carries w0 + late w1; ring B (scalar) carries x + early w1.
            w1_eng = [
                nc.scalar if s < n_slab - 2 else nc.sync
                for s in range(n_slab)
            ]
            for s, (ka, kb) in enumerate(bounds):
                ks = slice(ka, kb)
                t = w0pool.tile([128, kb - ka, n_half], mmdt, tag="w0", name=f"w0s{s}")
                nc.sync.dma_start(t[:], w0[:, ks, :])
                w0_slabs.append(t)
                t = xpool.tile([128, kb - ka, P], mmdt, tag="x", name=f"xs{s}")
                nc.scalar.dma_start(t[:], xt[:, ks, :])
                x_slabs.append(t)
                t = w1pool.tile([128, kb - ka, n_half], mmdt, tag="w1", name=f"w1s{s}")
                w1_eng[s].dma_start(t[:], w1[:, ks, :])
                w1_slabs.append(t)

            ps_tiles = [
                [
                    pspool.tile([128, n_half], f32, tag="ps", name=f"ps{h}_{i}")
                    for i in range(n_m)
                ]
                for h in range(2)
            ]
            w_halves = [w0_slabs, w1_slabs]
            k2slab = {}
            for s, (ka, kb) in enumerate(bounds):
                for k in range(ka, kb):
                    k2slab[k] = (s, k - ka)
            def mm(k, h, mi):
                s, j = k2slab[k]
                nc.tensor.matmul(
                    ps_tiles[h][mi][:, :],
                    x_slabs[s][:, j, mi * 128 : (mi + 1) * 128],
                    w_halves[h][s][:, j, :],
                    start=(k == 0),
                    stop=(k == n_k - 1),
                )

            # phase A: k-outer over all banks, tracks DMA arrival order
            for k in range(K_SPLIT):
                for mi in range(n_m):
                    for h in range(2):
                        mm(k, h, mi)
            # phase B: bank-major k-contiguous chains; copy+store each bank
            # as soon as its chain stops, overlapping the later chains
            for mi in range(n_m):
                for h in range(2):
                    for k in range(K_SPLIT, n_k):
                        mm(k, h, mi)
                    i = mi * 2 + h
                    ot = opool.tile([128, n_half], mmdt, tag="o", name=f"o{h}_{mi}")
                    nc.vector.tensor_copy(ot[:], ps_tiles[h][mi][:])
                    eng = nc.sync if i % 2 == 0 else nc.scalar
                    eng.dma_start(
                        y[
                            mi * 128 : (mi + 1) * 128,
                            h * n_half : (h + 1) * n_half,
                        ],
                        ot[:],
                    )

    nc.compile()
    return nc


def _build_v6(P, mode):
    """v5 + single fused input tensor: per k-tile, [x | w0half | w1half]
    packed as [128, n_k, P + 1024] bf16. Ramped slab DMAs alternating
    rings; all inputs stay resident in SBUF (48KB/partition).
    Slab 0 is split (x+w0 first, w1 behind) so the first MMs start on a
    256KB transfer. GMM_WARM dummy matmuls run during the DMA spin-up
    window to lift the PE out of its 1.2GHz cold p-state (first ~11 real
    MMs otherwise pace at 427ns instead of 216ns)."""
    import concourse.tile as tile
    from concourse import bacc, mybir

    f32 = mybir.dt.float32
    mmdt, _ = _dtypes(mode)
    assert mode == "bf16"

    n_k = K // 128  # 16
    n_m = P // 128  # 4
    n_half = N // 2  # 512
    W = N + P  # fused cols per k-tile

    RAMP = (1, 1, 2, 4, 4, 4)
    assert sum(RAMP) == n_k
    K_SPLIT = 12

    # cores run independently (no collectives): single-device NEFF
    # replicated per core skips cross-device setup in the preamble
    nc = bacc.Bacc(
        "TRN2", target_bir_lowering=False, debug=False, num_devices=1
    )

    a = nc.dram_tensor("a", [128, n_k, W], mmdt, kind="ExternalInput").ap()
    y = nc.dram_tensor("y", [P, N], mmdt, kind="ExternalOutput").ap()

    bounds = []
    k0 = 0
    for r in RAMP:
        bounds.append((k0, k0 + r))
        k0 += r
    k2slab = {}
    for s, (ka, kb) in enumerate(bounds):
        for k in range(ka, kb):
            k2slab[k] = (s, k - ka)

    import contextlib

    with tile.TileContext(nc) as tc:
        with (
            tc.tile_pool(name="a", bufs=len(RAMP) + 1) as apool,
            tc.tile_pool(name="ps", bufs=8, space="PSUM") as pspool,
            tc.tile_pool(name="o", bufs=8) as opool,
            (
                tc.tile_pool(name="wu", bufs=1)
                if WARM
                else contextlib.nullcontext()
            ) as wupool,
        ):
            # warm-up: memset a scratch tile, then WARM dummy N=128
            # matmuls keep the PE busy through the DMA spin-up window
            wu = None
            if WARM:
                wu = wupool.tile([128, 128], mmdt, tag="wu", name="wu")
                nc.gpsimd.memset(wu[:], 0)

            slabs = []  # per slab: (tile, xbase, wbase) col offsets
            for s, (ka, kb) in enumerate(bounds):
                if s == 0:
                    # split first slab: x+w0 cols on sync, w1 on scalar
                    t0 = apool.tile([128, 1, P + n_half], mmdt, tag="a", name="a0a")
                    nc.sync.dma_start(t0[:], a[:, 0:1, : P + n_half])
                    t1 = apool.tile([128, 1, n_half], mmdt, tag="a", name="a0b")
                    nc.scalar.dma_start(t1[:], a[:, 0:1, P + n_half :])
                    slabs.append((t0, t1))
                    continue
                t = apool.tile([128, kb - ka, W], mmdt, tag="a", name=f"a{s}")
                eng = nc.sync if s % 2 == 0 else nc.scalar
                eng.dma_start(t[:], a[:, ka:kb, :])
                slabs.append((t, None))

            ps_tiles = [
                [
                    pspool.tile([128, n_half], f32, tag="ps", name=f"ps{h}_{i}")
                    for i in range(n_m)
                ]
                for h in range(2)
            ]

            if WARM:
                # single accumulation chain -> back-to-back MMs, ramping
                # the PE clock 0.65 -> 2.4GHz during the DMA wait
                for i in range(WARM):
                    nc.tensor.matmul(
                        ps_tiles[0][0][:, 0:128],
                        wu[:],
                        wu[:],
                        start=(i == 0),
                        stop=(i == WARM - 1),
                    )

            def aps(k, h, mi):
                """(lhsT x-tile AP, moving w AP) for this k."""
                s, j = k2slab[k]
                t, t1 = slabs[s]
                if s == 0:
                    xap = t[:, 0, mi * 128 : (mi + 1) * 128]
                    if h == 0:
                        wap = t[:, 0, P : P + n_half]
                    else:
                        wap = t1[:, 0, 0:n_half]
                else:
                    xap = t[:, j, mi * 128 : (mi + 1) * 128]
                    wap = t[:, j, P + h * n_half : P + (h + 1) * n_half]
                return xap, wap

            def mm(k, h, mi):
                xap, wap = aps(k, h, mi)
                nc.tensor.matmul(
                    ps_tiles[h][mi][:, :],
                    xap,
                    wap,
                    start=(k == 0),
                    stop=(k == n_k - 1),
                )

            for k in range(K_SPLIT):
                for h in range(2):
                    for mi in range(n_m):
                        mm(k, h, mi)
            n_chain = n_m * 2
            for mi in range(n_m):
                for h in range(2):
                    for k in range(K_SPLIT, n_k):
                        mm(k, h, mi)
                    i = mi * 2 + h
                    ot = opool.tile([128, n_half], mmdt, tag="o", name=f"o{h}_{mi}")
                    ys = y[
                        mi * 128 : (mi + 1) * 128,
                        h * n_half : (h + 1) * n_half,
                    ]
                    if i < n_chain - 2:
                        nc.vector.tensor_copy(ot[:], ps_tiles[h][mi][:])
                        eng = nc.sync if i % 2 == 0 else nc.scalar
                        eng.dma_start(ys, ot[:])
                    else:
                        # final chains: half-column copies/stores shrink
                        # the exposed tail after the very last matmul
                        hw2 = n_half // 2
                        for c in range(2):
                            cs = slice(c * hw2, (c + 1) * hw2)
                            nc.vector.tensor_copy(
                                ot[:, cs], ps_tiles[h][mi][:, cs]
                            )
                            eng = nc.sync if (i + c) % 2 == 0 else nc.scalar
                            eng.dma_start(ys[:, cs], ot[:, cs])

    nc.compile()
    return nc


def _build_v3(P, mode):
    """k-outer over all PSUM banks, n-half waves for early output overlap,
    slab DMAs balanced across both HWDGE rings. P <= 1024."""
    import concourse.tile as tile
    from concourse import bacc, mybir

    f32 = mybir.dt.float32
    mmdt, _ = _dtypes(mode)
    two = mode == "bf16x3"

    KS = 2  # k-subtiles per DMA slab
    n_slab = K // (128 * KS)  # 8
    n_k = K // 128  # 16
    n_m = P // 128
    assert n_m <= 8
    n_half = N // 2  # 512: one psum bank per (m, half)

    nc = bacc.Bacc(
        "TRN2", target_bir_lowering=False, debug=False, num_devices=NCORES
    )

    def din(name, shape):
        return nc.dram_tensor(name, shape, mmdt, kind="ExternalInput").ap()

    y = nc.dram_tensor("y", [P, N], f32, kind="ExternalOutput").ap()
    if two:
        x_ins = [din("x_hi", [K, P]), din("x_lo", [K, P])]
        w_ins = [din("w_hi", [K, N]), din("w_lo", [K, N])]
    else:
        x_ins = [din("xT", [K, P])]
        w_ins = [din("wT", [K, N])]

    x_views = [a.rearrange("(po pi) f -> pi po f", pi=128) for a in x_ins]
    w_views = [a.rearrange("(po pi) f -> pi po f", pi=128) for a in w_ins]
    nw = len(w_ins)
    nx = len(x_ins)

    with tile.TileContext(nc) as tc:
        with (
            tc.tile_pool(name="w0", bufs=n_slab * nw) as w0pool,
            tc.tile_pool(name="w1", bufs=n_slab * nw) as w1pool,
            tc.tile_pool(name="x", bufs=n_slab * nx) as xpool,
            tc.tile_pool(name="ps", bufs=8, space="PSUM") as pspool,
            tc.tile_pool(name="o", bufs=8) as opool,
        ):
            # ring A (sync): w n-half 0 slabs; ring B (scalar): x slabs.
            # Then w n-half 1 slabs split across both rings.
            w0_slabs, w1_slabs, x_slabs = [], [], []
            for s in range(n_slab):
                ks = slice(s * KS, (s + 1) * KS)
                row = []
                for wv in w_views:
                    t = w0pool.tile([128, KS, n_half], mmdt, tag="w0")
                    nc.sync.dma_start(t[:], wv[:, ks, 0:n_half])
                    row.append(t)
                w0_slabs.append(row)
                row = []
                for xv in x_views:
                    t = xpool.tile([128, KS, P], mmdt, tag="x")
                    nc.scalar.dma_start(t[:], xv[:, ks, :])
                    row.append(t)
                x_slabs.append(row)
            for s in range(n_slab):
                ks = slice(s * KS, (s + 1) * KS)
                eng = nc.sync if s % 2 == 0 else nc.scalar
                row = []
                for wv in w_views:
                    t = w1pool.tile([128, KS, n_half], mmdt, tag="w1")
                    eng.dma_start(t[:], wv[:, ks, n_half:N])
                    row.append(t)
                w1_slabs.append(row)

            prods = [(0, 0)] if not two else [(0, 0), (1, 0), (0, 1)]
            n_acc = n_k * len(prods)

            def wave(w_slabs, ncol0, store_engines):
                ps_tiles = [
                    pspool.tile([128, n_half], f32, tag="ps", name=f"ps{i}")
                    for i in range(n_m)
                ]
                for k in range(n_k):
                    s, j = divmod(k, KS)
                    for mi in range(n_m):
                        i_acc0 = k * len(prods)
                        for pi, (xi, wi) in enumerate(prods):
                            nc.tensor.matmul(
                                ps_tiles[mi][:, :],
                                x_slabs[s][xi][:, j, mi * 128 : (mi + 1) * 128],
                                w_slabs[s][wi][:, j, :],
                                start=(i_acc0 + pi == 0),
                                stop=(i_acc0 + pi == n_acc - 1),
                            )
                for mi in range(n_m):
                    ot = opool.tile([128, n_half], f32, tag="o")
                    nc.vector.tensor_copy(ot[:], ps_tiles[mi][:])
                    eng = store_engines[mi % len(store_engines)]
                    eng.dma_start(
                        y[mi * 128 : (mi + 1) * 128, ncol0 : ncol0 + n_half], ot[:]
                    )

            # n-half 0 completes mid-stream; store via SWDGE to keep HWDGE
            # rings on input. n-half 1 stores at the end on the idle rings.
            wave(w0_slabs, 0, [nc.gpsimd])
            wave(w1_slabs, n_half, [nc.sync, nc.scalar])

    nc.compile()
    return nc


def _build_v2(P, mode):
    """Lean hand-rolled kernel: slab DMAs on both HWDGE rings, k-inner
    accumulation, outputs via SWDGE. P must be <= 1024."""
    import concourse.tile as tile
    from concourse import bacc, mybir

    f32 = mybir.dt.float32
    mmdt, _ = _dtypes(mode)
    two = mode == "bf16x3"

    KS = 4  # k-subtiles per DMA slab
    n_slab = K // (128 * KS)
    n_k = K // 128
    n_m = P // 128
    n_n = N // NT

    nc = bacc.Bacc(
        "TRN2", target_bir_lowering=False, debug=False, num_devices=NCORES
    )

    def din(name, shape):
        return nc.dram_tensor(name, shape, mmdt, kind="ExternalInput").ap()

    y = nc.dram_tensor("y", [P, N], f32, kind="ExternalOutput").ap()
    if two:
        x_ins = [din("x_hi", [K, P]), din("x_lo", [K, P])]
        w_ins = [din("w_hi", [K, N]), din("w_lo", [K, N])]
    else:
        x_ins = [din("xT", [K, P])]
        w_ins = [din("wT", [K, N])]

    x_views = [a.rearrange("(po pi) f -> pi po f", pi=128) for a in x_ins]
    w_views = [a.rearrange("(po pi) f -> pi po f", pi=128) for a in w_ins]

    with tile.TileContext(nc) as tc:
        with (
            tc.tile_pool(name="w", bufs=n_slab * len(w_ins)) as wpool,
            tc.tile_pool(name="x", bufs=n_slab * len(x_ins)) as xpool,
            tc.tile_pool(name="ps", bufs=4, space="PSUM") as pspool,
            tc.tile_pool(name="o", bufs=4) as opool,
        ):
            w_slabs, x_slabs = [], []
            for s in range(n_slab):
                ks = slice(s * KS, (s + 1) * KS)
                wrow, xrow = [], []
                for wi, wv in enumerate(w_views):
                    t = wpool.tile([128, KS, N], mmdt, tag="w")
                    nc.sync.dma_start(t[:], wv[:, ks, :])
                    wrow.append(t)
                for xi, xv in enumerate(x_views):
                    t = xpool.tile([128, KS, P], mmdt, tag="x")
                    nc.scalar.dma_start(t[:], xv[:, ks, :])
                    xrow.append(t)
                w_slabs.append(wrow)
                x_slabs.append(xrow)

            prods = [(0, 0)] if not two else [(0, 0), (1, 0), (0, 1)]
            n_acc = n_k * len(prods)
            for mi in range(n_m):
                ms = slice(mi * 128, (mi + 1) * 128)
                for ni in range(n_n):
                    nsl = slice(ni * NT, (ni + 1) * NT)
                    ps = pspool.tile([128, NT], f32, tag="ps")
                    i_acc = 0
                    for k in range(n_k):
                        s, j = divmod(k, KS)
                        for xi, wi in prods:
                            nc.tensor.matmul(
                                ps[:, :],
                                x_slabs[s][xi][:, j, ms],
                                w_slabs[s][wi][:, j, nsl],
                                start=(i_acc == 0),
                                stop=(i_acc == n_acc - 1),
                            )
                            i_acc += 1
                    ot = opool.tile([128, NT], f32, tag="o")
                    nc.vector.tensor_copy(ot[:], ps[:])
                    nc.gpsimd.dma_start(y[ms, nsl], ot[:])

    nc.compile()
    return nc


def _build(P, mode):
    import concourse.tile as tile
    from concourse import bacc, mybir
    from concourse.kernels.tile_matmul import matmul_tile_kernel

    f32 = mybir.dt.float32
    mmdt, _ = _dtypes(mode)
    two = mode == "bf16x3"  # hi/lo split inputs

    nc = bacc.Bacc(
        "TRN2", target_bir_lowering=False, debug=False, num_devices=NCORES
    )

    def din(name, shape):
        return nc.dram_tensor(name, shape, mmdt, kind="ExternalInput").ap()

    y = nc.dram_tensor("y", [P, N], f32, kind="ExternalOutput").ap()
    if two:
        x_hi, x_lo = din("x_hi", [K, P]), din("x_lo", [K, P])
        w_hi, w_lo = din("w_hi", [K, N]), din("w_lo", [K, N])
    else:
        xT, wT = din("xT", [K, P]), din("wT", [K, N])

    with tile.TileContext(nc) as tc:
        if two:
            # y = xhi.T@whi + xlo.T@whi + xhi.T@wlo, accumulated via DMA
            matmul_tile_kernel(tc, x_hi, w_hi, y)
            matmul_tile_kernel(tc, x_lo, w_hi, y, mxn_accum_op=mybir.AluOpType.add)
            matmul_tile_kernel(tc, x_hi, w_lo, y, mxn_accum_op=mybir.AluOpType.add)
        else:
            matmul_tile_kernel(tc, xT, wT, y)

    nc.compile()
    return nc


KERNEL_V = os.environ.get("GMM_KERNEL", "v6")


def _use_v6(P, mode):
    return KERNEL_V == "v6" and P <= 512 and mode == "bf16"


def _use_v5(P, mode):
    return KERNEL_V == "v5" and P <= 512 and mode == "bf16"


def _use_v4(P, mode):
    return KERNEL_V == "v4" and P <= 1024 and mode != "bf16x3"


def _get_nc(P, mode):
    key = (P, mode, KERNEL_V, CSPL, WARM)
    if key not in _nc_cache:
        if _use_v6(P, mode):
            _nc_cache[key] = _build_v6(P, mode)
        elif _use_v5(P, mode):
            _nc_cache[key] = _build_v5(P, mode)
        elif _use_v4(P, mode):
            _nc_cache[key] = _build_v4(P, mode)
        elif KERNEL_V in ("v3", "v4") and P <= 1024:
            _nc_cache[key] = _build_v3(P, mode)
        elif KERNEL_V == "v2" and P <= 1024:
            _nc_cache[key] = _build_v2(P, mode)
        else:
            _nc_cache[key] = _build(P, mode)
    return _nc_cache[key]


def _split_hi_lo(a, np_bf16):
    hi = a.astype(np_bf16)
    lo = (a - hi.astype(np.float32)).astype(np_bf16)
    return hi, lo


def kernel(x, weight, offs):
    global last_result
    from concourse.bass_utils import run_bass_kernel_spmd

    x = np.ascontiguousarray(x, dtype=np.float32)
    weight = np.ascontiguousarray(weight, dtype=np.float32)
    offs = np.asarray(offs, dtype=np.int64)

    starts = np.zeros(E, dtype=np.int64)
    starts[1:] = offs[:-1]
    starts = np.clip(starts, 0, T)
    ends = np.clip(offs, 0, T)
    sizes = np.maximum(ends - starts, 0)

    P = max(128, int(math.ceil(max(int(sizes.max()), 1) / 128.0)) * 128)
    mode = MODE
    _, np_in = _dtypes(mode)

    nc = _get_nc(P, mode)

    in_maps = []
    for e in range(E):
        xe = x[starts[e] : starts[e] + sizes[e]]
        xT = np.zeros((K, P), dtype=np.float32)
        xT[:, : sizes[e]] = xe.T
        wT = np.ascontiguousarray(weight[e].T)  # [K, N]
        if _use_v6(P, mode):
            # fused [128, n_k, P + N]: per k-tile [x cols | w cols]
            fused = np.empty((K, P + N), dtype=np.float32)
            fused[:, :P] = xT
            fused[:, P:] = wT
            a3 = np.ascontiguousarray(
                fused.reshape(K // 128, 128, P + N).transpose(1, 0, 2)
            ).astype(np_in)
            in_maps.append({"a": a3})
            continue
        if _use_v5(P, mode) or _use_v4(P, mode):
            # pre-tiled [pi, po, cols] layout, k = po*128 + pi
            def tile3(a):
                return np.ascontiguousarray(
                    a.reshape(K // 128, 128, a.shape[1]).transpose(1, 0, 2)
                ).astype(np_in)

            in_maps.append(
                {
                    "wT0": tile3(wT[:, : N // 2]),
                    "wT1": tile3(wT[:, N // 2 :]),
                    "xTt": tile3(xT),
                }
            )
            continue
        if mode == "bf16x3":
            import ml_dtypes

            bf = np.dtype(ml_dtypes.bfloat16)
            x_hi, x_lo = _split_hi_lo(xT, bf)
            w_hi, w_lo = _split_hi_lo(wT, bf)
            in_maps.append(
                {"x_hi": x_hi, "x_lo": x_lo, "w_hi": w_hi, "w_lo": w_lo}
            )
        elif mode == "bf16":
            in_maps.append({"xT": xT.astype(np_in), "wT": wT.astype(np_in)})
        else:
            in_maps.append({"xT": xT, "wT": wT})

    res = run_bass_kernel_spmd(
        nc, in_maps, list(range(NCORES)), trace=TRACE
    )
    last_result = res

    out = np.zeros((T, N), dtype=np.float32)
    for e in range(E):
        if sizes[e]:
            ye = np.asarray(res.results[e]["y"][: sizes[e]])
            out[starts[e] : ends[e]] = ye.astype(np.float32)
    return out



# revision 34
# speedup vs baseline: 1.1276x; 1.1276x over previous
"""Grouped matmul (MoE routing) kernel for Trainium2, 8 NeuronCores.

Problem: y[t] = x[t] @ weight[e].T for tokens t in [starts[e], offs[e]),
with x [4096, 2048] f32, weight [8, 1024, 2048] f32, offs [8] int32
(cumulative group ends). Output [4096, 1024] f32; tokens >= offs[-1] -> 0.

Strategy: expert-parallel. Routing is done host-side (offs is a host
numpy array): core e receives its expert's token slice, transposed and
zero-padded to P rows (x_e^T [K, P]), plus its expert's transposed
weight (w_e^T [K, N]). Each core runs a dense [P,K]x[K,N] matmul; the
host scatters per-core outputs back into the full [T, N] result.

Default path (v6, bf16): single fused pre-tiled input tensor
[128, K/128, P+N] per core ([x|w] per k-tile), ramped slab DMAs on both
HWDGE rings, k-outer accumulation over all 8 PSUM banks while the input
streams, then bank-major k-chains with immediate per-bank PSUM->SBUF
bf16 copy + store (host upcasts y to f32). Per-core floor is the PE
matmul stream: 128 MMs x 216ns = 27.7us (bf16 streams 1 col/cycle;
fp8 fails the 2e-2 accuracy gate, measured 3.8% rel err).

Matmul dtype modes (GMM_MODE env): fp32 (exact, 4 cyc/row), fp32r
(1 cyc/row at N=512), bf16 (1 cyc/row, half DMA), bf16x3 (hi/lo split,
near-fp32 accuracy, 3x bf16 compute).
"""

import math
import os
import sys

for _p in ("/opt/pypackages", "/opt/trn_rl_repo"):
    if _p not in sys.path:
        sys.path.insert(0, _p)

import numpy as np

E, K, N, T = 8, 2048, 1024, 4096
NCORES = 8
KT = 128  # contraction tile (PE partition dim)
NT = 512  # psum free-dim chunk (one PSUM bank of f32)
MB = 512  # m-block rows kept resident in SBUF at once

MODE = os.environ.get("GMM_MODE", "bf16")
TRACE = bool(int(os.environ.get("GMM_TRACE", "0")))
CSPL = int(os.environ.get("GMM_CSPL", "1"))  # column-group split in v6
WARM = int(os.environ.get("GMM_WARM", "0"))  # PE warm-up dummy matmuls

_nc_cache = {}
last_result = None  # BassKernelResults of the most recent run (for test.py)


def _dtypes(mode):
    from concourse import mybir

    if mode == "fp32":
        return mybir.dt.float32, np.float32
    if mode == "fp32r":
        return mybir.dt.float32r, np.float32
    import ml_dtypes

    return mybir.dt.bfloat16, np.dtype(ml_dtypes.bfloat16)


def _build_v4(P, mode):
    """v3 + host-pre-tiled inputs ([128, K/128, cols] layout -> 8KB DMA
    runs), KS=4, all stores on SWDGE. Single-tensor modes only."""
    import concourse.tile as tile
    from concourse import bacc, mybir

    f32 = mybir.dt.float32
    mmdt, _ = _dtypes(mode)

    KS = 4
    n_slab = K // (128 * KS)  # 4
    n_k = K // 128
    n_m = P // 128
    assert n_m <= 8
    n_half = N // 2

    nc = bacc.Bacc(
        "TRN2", target_bir_lowering=False, debug=False, num_devices=NCORES
    )

    w0 = nc.dram_tensor("wT0", [128, n_k, n_half], mmdt, kind="ExternalInput").ap()
    w1 = nc.dram_tensor("wT1", [128, n_k, n_half], mmdt, kind="ExternalInput").ap()
    xt = nc.dram_tensor("xTt", [128, n_k, P], mmdt, kind="ExternalInput").ap()
    y = nc.dram_tensor("y", [P, N], f32, kind="ExternalOutput").ap()

    WKS = 2  # w slab k-subtiles (finer pacing near stream end)
    n_wslab = K // (128 * WKS)

    with tile.TileContext(nc) as tc:
        with (
            tc.tile_pool(name="w0", bufs=n_wslab) as w0pool,
            tc.tile_pool(name="w1", bufs=n_wslab) as w1pool,
            tc.tile_pool(name="x", bufs=n_slab) as xpool,
            tc.tile_pool(name="ps", bufs=8, space="PSUM") as pspool,
            tc.tile_pool(name="o", bufs=8) as opool,
        ):
            # Balanced rings (~6.3MB each), w slabs arriving in k order:
            # ring A (sync): w0/w1 slabs k 0..11; ring B (scalar): x, then
            # w0/w1 slabs k 12..15 (the final MM chain's inputs).
            w0_slabs = [None] * n_wslab
            w1_slabs = [None] * n_wslab
            x_slabs = []
            for s in range(n_wslab - 2):
                ks = slice(s * WKS, (s + 1) * WKS)
                t = w0pool.tile([128, WKS, n_half], mmdt, tag="w0", name=f"w0s{s}")
                nc.sync.dma_start(t[:], w0[:, ks, :])
                w0_slabs[s] = t
                t = w1pool.tile([128, WKS, n_half], mmdt, tag="w1", name=f"w1s{s}")
                nc.sync.dma_start(t[:], w1[:, ks, :])
                w1_slabs[s] = t
            for s in range(n_slab):
                ks = slice(s * KS, (s + 1) * KS)
                t = xpool.tile([128, KS, P], mmdt, tag="x", name=f"xs{s}")
                nc.scalar.dma_start(t[:], xt[:, ks, :])
                x_slabs.append(t)
            for s in range(n_wslab - 2, n_wslab):
                ks = slice(s * WKS, (s + 1) * WKS)
                t = w0pool.tile([128, WKS, n_half], mmdt, tag="w0", name=f"w0s{s}")
                nc.scalar.dma_start(t[:], w0[:, ks, :])
                w0_slabs[s] = t
                t = w1pool.tile([128, WKS, n_half], mmdt, tag="w1", name=f"w1s{s}")
                nc.scalar.dma_start(t[:], w1[:, ks, :])
                w1_slabs[s] = t

            ps_tiles = [
                pspool.tile([128, n_half], f32, tag="ps", name=f"ps{h}_{i}")
                for h in range(2)
                for i in range(n_m)
            ]
            w_halves = [w0_slabs, w1_slabs]
            for k in range(n_k):
                ws, wj = divmod(k, WKS)
                xs, xj = divmod(k, KS)
                for h in range(2):
                    for mi in range(n_m):
                        nc.tensor.matmul(
                            ps_tiles[h * n_m + mi][:, :],
                            x_slabs[xs][:, xj, mi * 128 : (mi + 1) * 128],
                            w_halves[h][ws][:, wj, :],
                            start=(k == 0),
                            stop=(k == n_k - 1),
                        )
            for h in range(2):
                for mi in range(n_m):
                    ot = opool.tile(
                        [128, n_half], f32, tag="o", name=f"o{h}_{mi}"
                    )
                    nc.vector.tensor_copy(ot[:], ps_tiles[h * n_m + mi][:])
                    eng = nc.sync if (h * n_m + mi) % 2 == 0 else nc.scalar
                    eng.dma_start(
                        y[
                            mi * 128 : (mi + 1) * 128,
                            h * n_half : (h + 1) * n_half,
                        ],
                        ot[:],
                    )

    nc.compile()
    return nc


def _build_v5(P, mode):
    """bf16-first redesign from trace analysis of v4:
    - ramped slab sizes (tiny k0/k1 slabs -> early first MM, big tail slabs
      -> few DIRECT2D descriptor-gen ops, ~0.7us each on the sequencer)
    - k-outer / m-mid / h-inner MM order: consecutive MM pairs share the
      same stationary lhsT tile (chance for LDWEIGHTS elision) and slab
      consumption tracks DMA arrival order
    - output stored as bf16 (host upcasts): halves store bytes, DVE 2x
    - copies split DVE/ACT, stores split sync/scalar, emitted per-bank so
      stores overlap the final MMs of later banks."""
    import concourse.tile as tile
    from concourse import bacc, mybir

    f32 = mybir.dt.float32
    mmdt, _ = _dtypes(mode)
    assert mode == "bf16"

    n_k = K // 128  # 16
    n_m = P // 128  # 4
    assert n_m <= 4
    n_half = N // 2  # 512

    # slab ramp: k-tile counts per DMA slab
    RAMP = (1, 1, 2, 2, 4, 3, 3)
    assert sum(RAMP) == n_k
    K_SPLIT = 10  # phase A: k-outer 0..K_SPLIT-1; phase B: bank-major rest

    nc = bacc.Bacc(
        "TRN2", target_bir_lowering=False, debug=False, num_devices=NCORES
    )

    w0 = nc.dram_tensor("wT0", [128, n_k, n_half], mmdt, kind="ExternalInput").ap()
    w1 = nc.dram_tensor("wT1", [128, n_k, n_half], mmdt, kind="ExternalInput").ap()
    xt = nc.dram_tensor("xTt", [128, n_k, P], mmdt, kind="ExternalInput").ap()
    y = nc.dram_tensor("y", [P, N], mmdt, kind="ExternalOutput").ap()

    n_slab = len(RAMP)
    bounds = []
    k0 = 0
    for r in RAMP:
        bounds.append((k0, k0 + r))
        k0 += r

    with tile.TileContext(nc) as tc:
        with (
            tc.tile_pool(name="w0", bufs=n_slab) as w0pool,
            tc.tile_pool(name="w1", bufs=n_slab) as w1pool,
            tc.tile_pool(name="x", bufs=n_slab) as xpool,
            tc.tile_pool(name="ps", bufs=8, space="PSUM") as pspool,
            tc.tile_pool(name="o", bufs=8) as opool,
        ):
            w0_slabs, w1_slabs, x_slabs = [], [], []
            # interleaved issue tracking consumption order: ring A (sync)
            # carries w0 + late w1; ring B (scalar) carries x + early w1.
            w1_eng = [
                nc.scalar if s < n_slab - 2 else nc.sync
                for s in range(n_slab)
            ]
            for s, (ka, kb) in enumerate(bounds):
                ks = slice(ka, kb)
                t = w0pool.tile([128, kb - ka, n_half], mmdt, tag="w0", name=f"w0s{s}")
                nc.sync.dma_start(t[:], w0[:, ks, :])
                w0_slabs.append(t)
                t = xpool.tile([128, kb - ka, P], mmdt, tag="x", name=f"xs{s}")
                nc.scalar.dma_start(t[:], xt[:, ks, :])
                x_slabs.append(t)
                t = w1pool.tile([128, kb - ka, n_half], mmdt, tag="w1", name=f"w1s{s}")
                w1_eng[s].dma_start(t[:], w1[:, ks, :])
                w1_slabs.append(t)

            ps_tiles = [
                [
                    pspool.tile([128, n_half], f32, tag="ps", name=f"ps{h}_{i}")
                    for i in range(n_m)
                ]
                for h in range(2)
            ]
            w_halves = [w0_slabs, w1_slabs]
            k2slab = {}
            for s, (ka, kb) in enumerate(bounds):
                for k in range(ka, kb):
                    k2slab[k] = (s, k - ka)
            def mm(k, h, mi):
                s, j = k2slab[k]
                nc.tensor.matmul(
                    ps_tiles[h][mi][:, :],
                    x_slabs[s][:, j, mi * 128 : (mi + 1) * 128],
                    w_halves[h][s][:, j, :],
                    start=(k == 0),
                    stop=(k == n_k - 1),
                )

            # phase A: k-outer over all banks, tracks DMA arrival order
            for k in range(K_SPLIT):
                for mi in range(n_m):
                    for h in range(2):
                        mm(k, h, mi)
            # phase B: bank-major k-contiguous chains; copy+store each bank
            # as soon as its chain stops, overlapping the later chains
            for mi in range(n_m):
                for h in range(2):
                    for k in range(K_SPLIT, n_k):
                        mm(k, h, mi)
                    i = mi * 2 + h
                    ot = opool.tile([128, n_half], mmdt, tag="o", name=f"o{h}_{mi}")
                    nc.vector.tensor_copy(ot[:], ps_tiles[h][mi][:])
                    eng = nc.sync if i % 2 == 0 else nc.scalar
                    eng.dma_start(
                        y[
                            mi * 128 : (mi + 1) * 128,
                            h * n_half : (h + 1) * n_half,
                        ],
                        ot[:],
                    )

    nc.compile()
    return nc


def _build_v6(P, mode):
    """v5 + single fused input tensor: per k-tile, [x | w0half | w1half]
    packed as [128, n_k, P + 1024] bf16. Ramped slab DMAs alternating
    rings; all inputs stay resident in SBUF (48KB/partition).
    Slab 0 is split (x+w0 first, w1 behind) so the first MMs start on a
    256KB transfer. GMM_WARM dummy matmuls run during the DMA spin-up
    window to lift the PE out of its 1.2GHz cold p-state (first ~11 real
    MMs otherwise pace at 427ns instead of 216ns)."""
    import concourse.tile as tile
    from concourse import bacc, mybir

    f32 = mybir.dt.float32
    mmdt, _ = _dtypes(mode)
    assert mode == "bf16"

    n_k = K // 128  # 16
    n_m = P // 128  # 4
    n_half = N // 2  # 512
    W = N + P  # fused cols per k-tile

    RAMP = tuple(
        int(r) for r in os.environ.get("GMM_RAMP", "2,2,4,4,4").split(",")
    )
    assert sum(RAMP) == n_k
    K_SPLIT = int(os.environ.get("GMM_KSPLIT", "8"))

    # cores run independently (no collectives): single-device NEFF
    # replicated per core skips cross-device setup in the preamble
    nc = bacc.Bacc(
        "TRN2", target_bir_lowering=False, debug=False, num_devices=1
    )

    a = nc.dram_tensor("a", [128, n_k, W], mmdt, kind="ExternalInput").ap()
    y = nc.dram_tensor("y", [P, N], mmdt, kind="ExternalOutput").ap()

    bounds = []
    k0 = 0
    for r in RAMP:
        bounds.append((k0, k0 + r))
        k0 += r
    k2slab = {}
    for s, (ka, kb) in enumerate(bounds):
        for k in range(ka, kb):
            k2slab[k] = (s, k - ka)

    import contextlib

    with tile.TileContext(nc) as tc:
        with (
            tc.tile_pool(name="a", bufs=len(RAMP) + 1) as apool,
            tc.tile_pool(name="ps", bufs=8, space="PSUM") as pspool,
            tc.tile_pool(name="o", bufs=8) as opool,
            (
                tc.tile_pool(name="wu", bufs=1)
                if WARM
                else contextlib.nullcontext()
            ) as wupool,
        ):
            # warm-up: memset a scratch tile, then WARM dummy N=128
            # matmuls keep the PE busy through the DMA spin-up window
            wu = None
            if WARM:
                wu = wupool.tile([128, 128], mmdt, tag="wu", name="wu")
                nc.gpsimd.memset(wu[:], 0)

            slabs = []  # per slab: (tile, w1_tile_or_None)
            for s, (ka, kb) in enumerate(bounds):
                r = kb - ka
                if s == 0:
                    # split first slab: x+w0 cols on sync, w1 on scalar
                    t0 = apool.tile([128, r, P + n_half], mmdt, tag="a", name="a0a")
                    nc.sync.dma_start(t0[:], a[:, ka:kb, : P + n_half])
                    t1 = apool.tile([128, r, n_half], mmdt, tag="a", name="a0b")
                    nc.scalar.dma_start(t1[:], a[:, ka:kb, P + n_half :])
                    slabs.append((t0, t1))
                    continue
                t = apool.tile([128, r, W], mmdt, tag="a", name=f"a{s}")
                eng = nc.sync if s % 2 == 0 else nc.scalar
                eng.dma_start(t[:], a[:, ka:kb, :])
                slabs.append((t, None))

            ps_tiles = [
                [
                    pspool.tile([128, n_half], f32, tag="ps", name=f"ps{h}_{i}")
                    for i in range(n_m)
                ]
                for h in range(2)
            ]

            if WARM:
                # single accumulation chain -> back-to-back MMs, ramping
                # the PE clock 0.65 -> 2.4GHz during the DMA wait
                for i in range(WARM):
                    nc.tensor.matmul(
                        ps_tiles[0][0][:, 0:128],
                        wu[:],
                        wu[:],
                        start=(i == 0),
                        stop=(i == WARM - 1),
                    )

            def aps(k, h, mi):
                """(lhsT x-tile AP, moving w AP) for this k."""
                s, j = k2slab[k]
                t, t1 = slabs[s]
                xap = t[:, j, mi * 128 : (mi + 1) * 128]
                if s == 0:
                    if h == 0:
                        wap = t[:, j, P : P + n_half]
                    else:
                        wap = t1[:, j, 0:n_half]
                else:
                    wap = t[:, j, P + h * n_half : P + (h + 1) * n_half]
                return xap, wap

            def mm(k, h, mi):
                xap, wap = aps(k, h, mi)
                nc.tensor.matmul(
                    ps_tiles[h][mi][:, :],
                    xap,
                    wap,
                    start=(k == 0),
                    stop=(k == n_k - 1),
                )

            for k in range(K_SPLIT):
                for h in range(2):
                    for mi in range(n_m):
                        mm(k, h, mi)
            n_chain = n_m * 2
            for mi in range(n_m):
                for h in range(2):
                    for k in range(K_SPLIT, n_k):
                        mm(k, h, mi)
                    i = mi * 2 + h
                    ot = opool.tile([128, n_half], mmdt, tag="o", name=f"o{h}_{mi}")
                    ys = y[
                        mi * 128 : (mi + 1) * 128,
                        h * n_half : (h + 1) * n_half,
                    ]
                    if i < n_chain - 2:
                        nc.vector.tensor_copy(ot[:], ps_tiles[h][mi][:])
                        eng = nc.sync if i % 2 == 0 else nc.scalar
                        eng.dma_start(ys, ot[:])
                    else:
                        # final chains: half-column copies/stores shrink
                        # the exposed tail after the very last matmul
                        hw2 = n_half // 2
                        for c in range(2):
                            cs = slice(c * hw2, (c + 1) * hw2)
                            nc.vector.tensor_copy(
                                ot[:, cs], ps_tiles[h][mi][:, cs]
                            )
                            eng = nc.sync if (i + c) % 2 == 0 else nc.scalar
                            eng.dma_start(ys[:, cs], ot[:, cs])

    nc.compile()
    return nc


def _build_v7(P, mode):
    """v6 with 1024-wide moving operand: each matmul streams the full
    N=1024 w row-block into a 2-bank [128,1024] PSUM tile. Halves the
    tensor-queue instruction count (64 MMs) and makes output stores
    fully contiguous [128,1024] row blocks."""
    import contextlib

    import concourse.tile as tile
    from concourse import bacc, mybir

    f32 = mybir.dt.float32
    mmdt, _ = _dtypes(mode)
    assert mode == "bf16"

    n_k = K // 128  # 16
    n_m = P // 128  # 4
    W = N + P

    RAMP = (1, 1, 2, 4, 4, 4)
    assert sum(RAMP) == n_k
    K_SPLIT = 12

    nc = bacc.Bacc(
        "TRN2", target_bir_lowering=False, debug=False, num_devices=1
    )

    a = nc.dram_tensor("a", [128, n_k, W], mmdt, kind="ExternalInput").ap()
    y = nc.dram_tensor("y", [P, N], mmdt, kind="ExternalOutput").ap()

    bounds = []
    k0 = 0
    for r in RAMP:
        bounds.append((k0, k0 + r))
        k0 += r
    k2slab = {}
    for s, (ka, kb) in enumerate(bounds):
        for k in range(ka, kb):
            k2slab[k] = (s, k - ka)

    with tile.TileContext(nc) as tc:
        with (
            tc.tile_pool(name="a", bufs=len(RAMP)) as apool,
            tc.tile_pool(name="ps", bufs=4, space="PSUM") as pspool,
            tc.tile_pool(name="o", bufs=4) as opool,
        ):
            slabs = []
            for s, (ka, kb) in enumerate(bounds):
                t = apool.tile([128, kb - ka, W], mmdt, tag="a", name=f"a{s}")
                eng = nc.sync if s % 2 == 0 else nc.scalar
                eng.dma_start(t[:], a[:, ka:kb, :])
                slabs.append(t)

            ps_tiles = [
                pspool.tile([128, N], f32, tag="ps", name=f"ps{i}")
                for i in range(n_m)
            ]

            def mm(k, mi):
                s, j = k2slab[k]
                nc.tensor.matmul(
                    ps_tiles[mi][:, :],
                    slabs[s][:, j, mi * 128 : (mi + 1) * 128],
                    slabs[s][:, j, P : P + N],
                    start=(k == 0),
                    stop=(k == n_k - 1),
                )

            for k in range(K_SPLIT):
                for mi in range(n_m):
                    mm(k, mi)
            for mi in range(n_m):
                for k in range(K_SPLIT, n_k):
                    mm(k, mi)
                ot = opool.tile([128, N], mmdt, tag="o", name=f"o{mi}")
                ys = y[mi * 128 : (mi + 1) * 128, :]
                if mi < n_m - 2:
                    nc.vector.tensor_copy(ot[:], ps_tiles[mi][:])
                    eng = nc.sync if mi % 2 == 0 else nc.scalar
                    eng.dma_start(ys, ot[:])
                else:
                    # final chains: half-column copies/stores shrink the
                    # exposed tail after the very last matmul
                    for c in range(2):
                        cs = slice(c * (N // 2), (c + 1) * (N // 2))
                        nc.vector.tensor_copy(ot[:, cs], ps_tiles[mi][:, cs])
                        eng = nc.sync if (mi + c) % 2 == 0 else nc.scalar
                        eng.dma_start(ys[:, cs], ot[:, cs])

    nc.compile()
    return nc


def _build_v3(P, mode):
    """k-outer over all PSUM banks, n-half waves for early output overlap,
    slab DMAs balanced across both HWDGE rings. P <= 1024."""
    import concourse.tile as tile
    from concourse import bacc, mybir

    f32 = mybir.dt.float32
    mmdt, _ = _dtypes(mode)
    two = mode == "bf16x3"

    KS = 2  # k-subtiles per DMA slab
    n_slab = K // (128 * KS)  # 8
    n_k = K // 128  # 16
    n_m = P // 128
    assert n_m <= 8
    n_half = N // 2  # 512: one psum bank per (m, half)

    nc = bacc.Bacc(
        "TRN2", target_bir_lowering=False, debug=False, num_devices=NCORES
    )

    def din(name, shape):
        return nc.dram_tensor(name, shape, mmdt, kind="ExternalInput").ap()

    y = nc.dram_tensor("y", [P, N], f32, kind="ExternalOutput").ap()
    if two:
        x_ins = [din("x_hi", [K, P]), din("x_lo", [K, P])]
        w_ins = [din("w_hi", [K, N]), din("w_lo", [K, N])]
    else:
        x_ins = [din("xT", [K, P])]
        w_ins = [din("wT", [K, N])]

    x_views = [a.rearrange("(po pi) f -> pi po f", pi=128) for a in x_ins]
    w_views = [a.rearrange("(po pi) f -> pi po f", pi=128) for a in w_ins]
    nw = len(w_ins)
    nx = len(x_ins)

    with tile.TileContext(nc) as tc:
        with (
            tc.tile_pool(name="w0", bufs=n_slab * nw) as w0pool,
            tc.tile_pool(name="w1", bufs=n_slab * nw) as w1pool,
            tc.tile_pool(name="x", bufs=n_slab * nx) as xpool,
            tc.tile_pool(name="ps", bufs=8, space="PSUM") as pspool,
            tc.tile_pool(name="o", bufs=8) as opool,
        ):
            # ring A (sync): w n-half 0 slabs; ring B (scalar): x slabs.
            # Then w n-half 1 slabs split across both rings.
            w0_slabs, w1_slabs, x_slabs = [], [], []
            for s in range(n_slab):
                ks = slice(s * KS, (s + 1) * KS)
                row = []
                for wv in w_views:
                    t = w0pool.tile([128, KS, n_half], mmdt, tag="w0")
                    nc.sync.dma_start(t[:], wv[:, ks, 0:n_half])
                    row.append(t)
                w0_slabs.append(row)
                row = []
                for xv in x_views:
                    t = xpool.tile([128, KS, P], mmdt, tag="x")
                    nc.scalar.dma_start(t[:], xv[:, ks, :])
                    row.append(t)
                x_slabs.append(row)
            for s in range(n_slab):
                ks = slice(s * KS, (s + 1) * KS)
                eng = nc.sync if s % 2 == 0 else nc.scalar
                row = []
                for wv in w_views:
                    t = w1pool.tile([128, KS, n_half], mmdt, tag="w1")
                    eng.dma_start(t[:], wv[:, ks, n_half:N])
                    row.append(t)
                w1_slabs.append(row)

            prods = [(0, 0)] if not two else [(0, 0), (1, 0), (0, 1)]
            n_acc = n_k * len(prods)

            def wave(w_slabs, ncol0, store_engines):
                ps_tiles = [
                    pspool.tile([128, n_half], f32, tag="ps", name=f"ps{i}")
                    for i in range(n_m)
                ]
                for k in range(n_k):
                    s, j = divmod(k, KS)
                    for mi in range(n_m):
                        i_acc0 = k * len(prods)
                        for pi, (xi, wi) in enumerate(prods):
                            nc.tensor.matmul(
                                ps_tiles[mi][:, :],
                                x_slabs[s][xi][:, j, mi * 128 : (mi + 1) * 128],
                                w_slabs[s][wi][:, j, :],
                                start=(i_acc0 + pi == 0),
                                stop=(i_acc0 + pi == n_acc - 1),
                            )
                for mi in range(n_m):
                    ot = opool.tile([128, n_half], f32, tag="o")
                    nc.vector.tensor_copy(ot[:], ps_tiles[mi][:])
                    eng = store_engines[mi % len(store_engines)]
                    eng.dma_start(
                        y[mi * 128 : (mi + 1) * 128, ncol0 : ncol0 + n_half], ot[:]
                    )

            # n-half 0 completes mid-stream; store via SWDGE to keep HWDGE
            # rings on input. n-half 1 stores at the end on the idle rings.
            wave(w0_slabs, 0, [nc.gpsimd])
            wave(w1_slabs, n_half, [nc.sync, nc.scalar])

    nc.compile()
    return nc


def _build_v2(P, mode):
    """Lean hand-rolled kernel: slab DMAs on both HWDGE rings, k-inner
    accumulation, outputs via SWDGE. P must be <= 1024."""
    import concourse.tile as tile
    from concourse import bacc, mybir

    f32 = mybir.dt.float32
    mmdt, _ = _dtypes(mode)
    two = mode == "bf16x3"

    KS = 4  # k-subtiles per DMA slab
    n_slab = K // (128 * KS)
    n_k = K // 128
    n_m = P // 128
    n_n = N // NT

    nc = bacc.Bacc(
        "TRN2", target_bir_lowering=False, debug=False, num_devices=NCORES
    )

    def din(name, shape):
        return nc.dram_tensor(name, shape, mmdt, kind="ExternalInput").ap()

    y = nc.dram_tensor("y", [P, N], f32, kind="ExternalOutput").ap()
    if two:
        x_ins = [din("x_hi", [K, P]), din("x_lo", [K, P])]
        w_ins = [din("w_hi", [K, N]), din("w_lo", [K, N])]
    else:
        x_ins = [din("xT", [K, P])]
        w_ins = [din("wT", [K, N])]

    x_views = [a.rearrange("(po pi) f -> pi po f", pi=128) for a in x_ins]
    w_views = [a.rearrange("(po pi) f -> pi po f", pi=128) for a in w_ins]

    with tile.TileContext(nc) as tc:
        with (
            tc.tile_pool(name="w", bufs=n_slab * len(w_ins)) as wpool,
            tc.tile_pool(name="x", bufs=n_slab * len(x_ins)) as xpool,
            tc.tile_pool(name="ps", bufs=4, space="PSUM") as pspool,
            tc.tile_pool(name="o", bufs=4) as opool,
        ):
            w_slabs, x_slabs = [], []
            for s in range(n_slab):
                ks = slice(s * KS, (s + 1) * KS)
                wrow, xrow = [], []
                for wi, wv in enumerate(w_views):
                    t = wpool.tile([128, KS, N], mmdt, tag="w")
                    nc.sync.dma_start(t[:], wv[:, ks, :])
                    wrow.append(t)
                for xi, xv in enumerate(x_views):
                    t = xpool.tile([128, KS, P], mmdt, tag="x")
                    nc.scalar.dma_start(t[:], xv[:, ks, :])
                    xrow.append(t)
                w_slabs.append(wrow)
                x_slabs.append(xrow)

            prods = [(0, 0)] if not two else [(0, 0), (1, 0), (0, 1)]
            n_acc = n_k * len(prods)
            for mi in range(n_m):
                ms = slice(mi * 128, (mi + 1) * 128)
                for ni in range(n_n):
                    nsl = slice(ni * NT, (ni + 1) * NT)
                    ps = pspool.tile([128, NT], f32, tag="ps")
                    i_acc = 0
                    for k in range(n_k):
                        s, j = divmod(k, KS)
                        for xi, wi in prods:
                            nc.tensor.matmul(
                                ps[:, :],
                                x_slabs[s][xi][:, j, ms],
                                w_slabs[s][wi][:, j, nsl],
                                start=(i_acc == 0),
                                stop=(i_acc == n_acc - 1),
                            )
                            i_acc += 1
                    ot = opool.tile([128, NT], f32, tag="o")
                    nc.vector.tensor_copy(ot[:], ps[:])
                    nc.gpsimd.dma_start(y[ms, nsl], ot[:])

    nc.compile()
    return nc


def _build(P, mode):
    import concourse.tile as tile
    from concourse import bacc, mybir
    from concourse.kernels.tile_matmul import matmul_tile_kernel

    f32 = mybir.dt.float32
    mmdt, _ = _dtypes(mode)
    two = mode == "bf16x3"  # hi/lo split inputs

    nc = bacc.Bacc(
        "TRN2", target_bir_lowering=False, debug=False, num_devices=NCORES
    )

    def din(name, shape):
        return nc.dram_tensor(name, shape, mmdt, kind="ExternalInput").ap()

    y = nc.dram_tensor("y", [P, N], f32, kind="ExternalOutput").ap()
    if two:
        x_hi, x_lo = din("x_hi", [K, P]), din("x_lo", [K, P])
        w_hi, w_lo = din("w_hi", [K, N]), din("w_lo", [K, N])
    else:
        xT, wT = din("xT", [K, P]), din("wT", [K, N])

    with tile.TileContext(nc) as tc:
        if two:
            # y = xhi.T@whi + xlo.T@whi + xhi.T@wlo, accumulated via DMA
            matmul_tile_kernel(tc, x_hi, w_hi, y)
            matmul_tile_kernel(tc, x_lo, w_hi, y, mxn_accum_op=mybir.AluOpType.add)
            matmul_tile_kernel(tc, x_hi, w_lo, y, mxn_accum_op=mybir.AluOpType.add)
        else:
            matmul_tile_kernel(tc, xT, wT, y)

    nc.compile()
    return nc


KERNEL_V = os.environ.get("GMM_KERNEL", "v6")


def _use_v7(P, mode):
    return KERNEL_V == "v7" and P <= 512 and mode == "bf16"


def _use_v6(P, mode):
    return KERNEL_V in ("v6", "v7") and P <= 512 and mode == "bf16"


def _use_v5(P, mode):
    return KERNEL_V == "v5" and P <= 512 and mode == "bf16"


def _use_v4(P, mode):
    return KERNEL_V == "v4" and P <= 1024 and mode != "bf16x3"


def _get_nc(P, mode):
    key = (P, mode, KERNEL_V, CSPL, WARM)
    if key not in _nc_cache:
        if _use_v7(P, mode):
            _nc_cache[key] = _build_v7(P, mode)
        elif _use_v6(P, mode):
            _nc_cache[key] = _build_v6(P, mode)
        elif _use_v5(P, mode):
            _nc_cache[key] = _build_v5(P, mode)
        elif _use_v4(P, mode):
            _nc_cache[key] = _build_v4(P, mode)
        elif KERNEL_V in ("v3", "v4") and P <= 1024:
            _nc_cache[key] = _build_v3(P, mode)
        elif KERNEL_V == "v2" and P <= 1024:
            _nc_cache[key] = _build_v2(P, mode)
        else:
            _nc_cache[key] = _build(P, mode)
    return _nc_cache[key]


def _split_hi_lo(a, np_bf16):
    hi = a.astype(np_bf16)
    lo = (a - hi.astype(np.float32)).astype(np_bf16)
    return hi, lo


def kernel(x, weight, offs):
    global last_result
    from concourse.bass_utils import run_bass_kernel_spmd

    x = np.ascontiguousarray(x, dtype=np.float32)
    weight = np.ascontiguousarray(weight, dtype=np.float32)
    offs = np.asarray(offs, dtype=np.int64)

    starts = np.zeros(E, dtype=np.int64)
    starts[1:] = offs[:-1]
    starts = np.clip(starts, 0, T)
    ends = np.clip(offs, 0, T)
    sizes = np.maximum(ends - starts, 0)

    P = max(128, int(math.ceil(max(int(sizes.max()), 1) / 128.0)) * 128)
    mode = MODE
    _, np_in = _dtypes(mode)

    nc = _get_nc(P, mode)

    in_maps = []
    for e in range(E):
        xe = x[starts[e] : starts[e] + sizes[e]]
        xT = np.zeros((K, P), dtype=np.float32)
        xT[:, : sizes[e]] = xe.T
        wT = np.ascontiguousarray(weight[e].T)  # [K, N]
        if _use_v6(P, mode):
            # fused [128, n_k, P + N]: per k-tile [x cols | w cols]
            fused = np.empty((K, P + N), dtype=np.float32)
            fused[:, :P] = xT
            fused[:, P:] = wT
            a3 = np.ascontiguousarray(
                fused.reshape(K // 128, 128, P + N).transpose(1, 0, 2)
            ).astype(np_in)
            in_maps.append({"a": a3})
            continue
        if _use_v5(P, mode) or _use_v4(P, mode):
            # pre-tiled [pi, po, cols] layout, k = po*128 + pi
            def tile3(a):
                return np.ascontiguousarray(
                    a.reshape(K // 128, 128, a.shape[1]).transpose(1, 0, 2)
                ).astype(np_in)

            in_maps.append(
                {
                    "wT0": tile3(wT[:, : N // 2]),
                    "wT1": tile3(wT[:, N // 2 :]),
                    "xTt": tile3(xT),
                }
            )
            continue
        if mode == "bf16x3":
            import ml_dtypes

            bf = np.dtype(ml_dtypes.bfloat16)
            x_hi, x_lo = _split_hi_lo(xT, bf)
            w_hi, w_lo = _split_hi_lo(wT, bf)
            in_maps.append(
                {"x_hi": x_hi, "x_lo": x_lo, "w_hi": w_hi, "w_lo": w_lo}
            )
        elif mode == "bf16":
            in_maps.append({"xT": xT.astype(np_in), "wT": wT.astype(np_in)})
        else:
            in_maps.append({"xT": xT, "wT": wT})

    res = run_bass_kernel_spmd(
        nc, in_maps, list(range(NCORES)), trace=TRACE
    )
    last_result = res

    out = np.zeros((T, N), dtype=np.float32)
    for e in range(E):
        if sizes[e]:
            ye = np.asarray(res.results[e]["y"][: sizes[e]])
            out[starts[e] : ends[e]] = ye.astype(np.float32)
    return out

